# revision 10
# baseline (speedup 1.0000x reference)
"""Trainium2 Bass kernel for nn_ClassConditionalCurvature (segment_reduce).

Strategy (8 NeuronCores, SPMD, one data-independent program):
  Host: sort rows by class label, pad every class block to 1024 rows with
  marker embeddings [1e18, 0, ...] (their huge squared norm self-masks them
  out of the top-k, and their one-hot rows are zero so they contribute
  nothing to any segment sum).  Core k owns class k's full 1024-row block
  (k = 0..7) plus chunk k (128 rows) of classes 8 and 9.  Because each core
  holds the *full* column blocks of exactly the classes its rows belong to,
  every per-row quantity (intra-class kNN distances, distance-to-centroid)
  is computed entirely locally -- no collectives.
  Device per 128-row segment: y = -d^2 = 2*(G - sqn/2) - sqm via PE matmuls
  (column norms folded in as a K=1 ones-row matmul), 11 smallest distances
  via max8 -> match_replace -> max8 on the vector engine, sqrt on just the
  10 kept values, then per-class segment sums via small one-hot matmuls.
  Host: assemble the 10x256 centroids / 10 spreads / 10x10 local stats from
  the per-core partials and evaluate the tiny scalar loss terms in numpy.
"""

import numpy as np

import concourse.bacc as bacc
import concourse.bass as bass
import concourse.mybir as mybir
import concourse.tile as tile
from concourse import bass_utils
from concourse.masks import make_identity

F32 = mybir.dt.float32
AF = mybir.ActivationFunctionType
ALU = mybir.AluOpType
AX = mybir.AxisListType

N_CORES = 8
N, D = 8192, 256
C = 10
K = 10  # neighbors
EPS = 1e-8
CENTROID_W, SPREAD_W, LOCAL_W = 1.0, 0.5, 0.5

PADW = 1024          # padded class-block width (rows/cols per class)
NCH = PADW // 128    # chunks per block = 8
NSEG = 10            # row segments per core: 8 (block A) + 1 (B chunk) + 1 (C chunk)
PAD_MARK = 2.0e17    # pad embedding: [PAD_MARK, 0, ..., 0]; pad d^2 ~ 4e34 stays
                     # inside the scalar-engine sqrt domain [0, 2^118]
NEG_BIG = -3.0e38    # match_replace fill


def build_program():
    nc = bacc.Bacc("TRN2", target_bir_lowering=False, debug=False,
                   num_devices=N_CORES)
    # ---- I/O ----
    colsA = nc.dram_tensor("colsA", [PADW, D], F32, kind="ExternalInput")
    colsB = nc.dram_tensor("colsB", [PADW, D], F32, kind="ExternalInput")
    colsC = nc.dram_tensor("colsC", [PADW, D], F32, kind="ExternalInput")
    rowsBC = nc.dram_tensor("rowsBC", [256, D], F32, kind="ExternalInput")
    oh3_d = nc.dram_tensor("oh3", [NSEG * 128, 3], F32, kind="ExternalInput")
    npBC_d = nc.dram_tensor("npBC", [2 * PADW], F32, kind="ExternalInput")
    rc3_d = nc.dram_tensor("rc3", [3], F32, kind="ExternalInput")
    out_cent = nc.dram_tensor("out_cent", [3, D], F32, kind="ExternalOutput")
    out_spread = nc.dram_tensor("out_spread", [3, 1], F32, kind="ExternalOutput")
    out_local = nc.dram_tensor("out_local", [3, K], F32, kind="ExternalOutput")

    blocks = [colsA, colsB, colsC]

    with tile.TileContext(nc) as tc:
        with (
            tc.tile_pool(name="const", bufs=1) as const,
            tc.tile_pool(name="embt", bufs=1) as embt,
            tc.tile_pool(name="chunks", bufs=4) as chunks,
            tc.tile_pool(name="scratch", bufs=2) as scratch,
            tc.tile_pool(name="ybuf", bufs=2) as ybuf,
            tc.tile_pool(name="small", bufs=3) as small,
            tc.tile_pool(name="tpp", bufs=2, space="PSUM") as tpp,
            tc.tile_pool(name="csump", bufs=1, space="PSUM") as csump,
            tc.tile_pool(name="centtp", bufs=1, space="PSUM") as centtp,
            tc.tile_pool(name="accp", bufs=1, space="PSUM") as accp,
            tc.tile_pool(name="dotp", bufs=1, space="PSUM") as dotp,
            tc.tile_pool(name="gp", bufs=2, space="PSUM") as gp,
        ):
            # ---- constants ----
            ident = const.tile([128, 128], F32)
            make_identity(nc, ident[:])
            ones = const.tile([1, 128], F32)
            nc.vector.memset(ones[:], 1.0)
            eps_t = const.tile([128, 1], F32)
            nc.vector.memset(eps_t[:], EPS)
            oh3 = const.tile([128, NSEG, 3], F32)
            nc.sync.dma_start(oh3[:], oh3_d.ap().rearrange("(c p) f -> p c f", p=128))
            npBC = const.tile([128, 2, NCH], F32)
            nc.sync.dma_start(npBC[:], npBC_d.ap().rearrange("(b c p) -> p b c", b=2, p=128))
            rc3 = const.tile([1, 3], F32)
            nc.sync.dma_start(rc3[:], rc3_d.ap()[None, :])

            # persistent sbuf tensors
            embT = [embt.tile([128, 2 * PADW], F32, tag=f"embT{x}",
                              name=f"embT{x}") for x in range(3)]
            sqT = [embt.tile([1, PADW], F32, tag=f"sqT{x}", name=f"sqT{x}")
                   for x in range(3)]
            sqB = [embt.tile([128, NCH], F32, tag=f"sqB{x}", name=f"sqB{x}")
                   for x in range(3)]
            sqR = embt.tile([128, 2], F32)          # rowsBC squared norms
            embT_rows = embt.tile([128, 4, 128], F32)  # [p, half*2+j, f]
            negsq = embt.tile([128, NSEG], F32)
            cents = embt.tile([1, 3, D], F32)
            csq3 = embt.tile([1, 3], F32)
            k3 = embt.tile([1, 3], F32)
            centT = embt.tile([128, 6], F32)        # cols 0-2: d<128, 3-5: d>=128

            # PSUM accumulators (one open accumulation group per bank)
            csum_p = csump.tile([1, 256], F32)     # per-block centroid sum (3 sequential groups)
            centT_p = centtp.tile([128, 6], F32)   # 6 single-matmul transposed-centroid cols
            acc_p = accp.tile([3, 1 + K], F32)     # col 0: spread sums, cols 1..10: local sums

            # ---- setup: stream blocks, transpose, norms, centroid sums ----
            for bx in range(3):
                for ci in range(NCH):
                    ch = chunks.tile([128, D], F32)
                    nc.sync.dma_start(ch[:], blocks[bx].ap()[ci * 128:(ci + 1) * 128, :])
                    sc = scratch.tile([128, D], F32)
                    nc.scalar.activation(sc[:], ch[:], AF.Square,
                                         accum_out=sqB[bx][:, ci:ci + 1])
                    tp = tpp.tile([128, 384], F32)
                    nc.tensor.transpose(tp[:, 0:128], ch[:, 0:128], ident[:])
                    nc.tensor.transpose(tp[:, 128:256], ch[:, 128:256], ident[:])
                    nc.tensor.transpose(tp[0:1, 256:384], sqB[bx][:, ci:ci + 1], ident[:])
                    # psum -> embT (lo half to cols [ci*128], hi half to [PADW + ci*128])
                    ev = embT[bx][:].rearrange("p (h w) -> p h w", h=2)
                    nc.vector.tensor_copy(
                        ev[:, :, ci * 128:(ci + 1) * 128],
                        tp[:, 0:256].rearrange("p (h w) -> p h w", h=2))
                    nc.scalar.activation(sqT[bx][0:1, ci * 128:(ci + 1) * 128],
                                         tp[0:1, 256:384], AF.Copy, scale=-0.5)
                    # centroid partial sum over non-pad rows
                    if bx == 0:
                        notpad = oh3[:, ci, 0:1]
                    else:
                        notpad = npBC[:, bx - 1, ci:ci + 1]
                    nc.tensor.matmul(csum_p[:], notpad, ch[:],
                                     start=(ci == 0), stop=(ci == NCH - 1))
                # centroid for this block
                cent_b = cents[:, bx, :]
                nc.scalar.activation(cent_b, csum_p[:], AF.Copy,
                                     scale=rc3[0:1, bx:bx + 1])
                nc.sync.dma_start(out_cent.ap()[bx:bx + 1, :], cent_b)
                sc2 = scratch.tile([1, D], F32, tag="sc2")
                nc.scalar.activation(sc2[:], cent_b, AF.Square,
                                     accum_out=csq3[0:1, bx:bx + 1])
                nc.tensor.matmul(centT_p[:, bx:bx + 1],
                                 cents[0:1, bx, 0:128], ones[0:1, 0:1],
                                 start=True, stop=True)
                nc.tensor.matmul(centT_p[:, 3 + bx:4 + bx],
                                 cents[0:1, bx, 128:256], ones[0:1, 0:1],
                                 start=True, stop=True)

            # rowsBC chunks (rows of classes 8/9 owned by this core)
            for j in range(2):
                ch = chunks.tile([128, D], F32)
                nc.sync.dma_start(ch[:], rowsBC.ap()[j * 128:(j + 1) * 128, :])
                sc = scratch.tile([128, D], F32)
                nc.scalar.activation(sc[:], ch[:], AF.Square,
                                     accum_out=sqR[:, j:j + 1])
                tp = tpp.tile([128, 384], F32)
                nc.tensor.transpose(tp[:, 0:128], ch[:, 0:128], ident[:])
                nc.tensor.transpose(tp[:, 128:256], ch[:, 128:256], ident[:])
                nc.vector.tensor_copy(embT_rows[:, j, :], tp[:, 0:128])
                nc.vector.tensor_copy(embT_rows[:, 2 + j, :], tp[:, 128:256])

            nc.scalar.mul(k3[:], csq3[:], -0.5)
            nc.vector.tensor_copy(centT[:], centT_p[:])
            nc.scalar.mul(negsq[:, 0:NCH], sqB[0][:], -1.0)
            nc.scalar.mul(negsq[:, NCH:NSEG], sqR[:], -1.0)

            # ---- main loop: one 128-row segment at a time ----
            for seg in range(NSEG):
                if seg < NCH:
                    lo = embT[0][:, seg * 128:(seg + 1) * 128]
                    hi = embT[0][:, PADW + seg * 128:PADW + (seg + 1) * 128]
                    cT, sT = embT[0], sqT[0]
                    sqrow = sqB[0][:, seg:seg + 1]
                else:
                    j = seg - NCH
                    lo = embT_rows[:, j, :]
                    hi = embT_rows[:, 2 + j, :]
                    cT, sT = embT[1 + j], sqT[1 + j]
                    sqrow = sqR[:, j:j + 1]
                ohs = oh3[:, seg, :]

                y = ybuf.tile([128, PADW], F32)
                for h in range(2):
                    fs = h * 512
                    g = gp.tile([128, 512], F32)
                    nc.tensor.matmul(g[:], lo, cT[:, fs:fs + 512],
                                     start=True, stop=False)
                    nc.tensor.matmul(g[:], hi, cT[:, PADW + fs:PADW + fs + 512],
                                     start=False, stop=False)
                    nc.tensor.matmul(g[:], ones[:], sT[0:1, fs:fs + 512],
                                     start=False, stop=True)
                    # y = 2*G - sqm   (= -d^2 since G already holds e.e - sqn/2)
                    nc.scalar.activation(y[:, fs:fs + 512], g[:], AF.Identity,
                                         bias=negsq[:, seg:seg + 1], scale=2.0)

                m1 = small.tile([128, 8], F32, tag="m1")
                nc.vector.max(m1[:], y[:])
                nc.vector.match_replace(y[:], m1[:], y[:], NEG_BIG)
                m2 = small.tile([128, 8], F32, tag="m2")
                nc.vector.max(m2[:], y[:])

                knn = small.tile([128, K], F32, tag="knn")
                nc.scalar.activation(knn[:, 0:7], m1[:, 1:8], AF.Sqrt, scale=-1.0)
                nc.scalar.activation(knn[:, 7:10], m2[:, 0:3], AF.Sqrt, scale=-1.0)
                ksum = small.tile([128, 1], F32, tag="ksum")
                nc.vector.reduce_sum(ksum[:], knn[:], axis=AX.X)
                meps = small.tile([128, 1], F32, tag="meps")
                nc.scalar.activation(meps[:], ksum[:], AF.Identity,
                                     bias=eps_t[:], scale=1.0 / K)
                rmean = small.tile([128, 1], F32, tag="rmean")
                nc.vector.reciprocal(rmean[:], meps[:])
                # knd: col 0 = d2c, cols 1..10 = knn_norm -> one segment-sum matmul
                knd = small.tile([128, 1 + K], F32, tag="knd")
                nc.vector.tensor_scalar_mul(knd[:, 1:1 + K], knn[:], rmean[:])

                dt = dotp.tile([128, 3], F32)
                nc.tensor.matmul(dt[:], lo, centT[:, 0:3], start=True, stop=False)
                nc.tensor.matmul(dt[:], hi, centT[:, 3:6], start=False, stop=False)
                nc.tensor.matmul(dt[:], ones[:], k3[:], start=False, stop=True)
                # (tensor_tensor_reduce hard-faults TRN2 here; use mul+reduce)
                scr3 = small.tile([128, 3], F32, tag="scr3")
                sel = small.tile([128, 1], F32, tag="sel")
                nc.vector.tensor_mul(scr3[:], ohs, dt[:])
                nc.vector.reduce_sum(sel[:], scr3[:], axis=AX.X)
                nc.scalar.activation(knd[:, 0:1], sel[:], AF.Sqrt,
                                     bias=sqrow, scale=-2.0)

                nc.tensor.matmul(acc_p[:], ohs, knd[:],
                                 start=(seg == 0), stop=(seg == NSEG - 1))

            # ---- epilogue ----
            acc_sb = small.tile([3, 1 + K], F32, tag="acc_sb")
            nc.vector.tensor_copy(acc_sb[:], acc_p[:])
            nc.sync.dma_start(out_spread.ap(), acc_sb[:, 0:1])
            nc.sync.dma_start(out_local.ap(), acc_sb[:, 1:1 + K])

    nc.compile()
    return nc


_NC_CACHE = None


def _get_program():
    global _NC_CACHE
    if _NC_CACHE is None:
        _NC_CACHE = build_program()
    return _NC_CACHE


def make_core_inputs(embeddings, labels):
    """Host-side sharding: returns (in_maps list per core, counts)."""
    emb = np.ascontiguousarray(np.asarray(embeddings), dtype=np.float32)
    lab = np.asarray(labels).astype(np.int64).ravel()
    n = emb.shape[0]
    counts = np.bincount(lab, minlength=C)
    assert counts.max() <= PADW, f"class too large: {counts.max()} > {PADW}"
    assert counts.min() >= K + 2, f"class too small for kNN: {counts.min()}"

    blocks = np.zeros((C, PADW, D), dtype=np.float32)
    blocks[:, :, 0] = PAD_MARK
    order = np.argsort(lab, kind="stable")
    off = 0
    for c in range(C):
        idx = order[off:off + counts[c]]
        blocks[c, :counts[c]] = emb[idx]
        off += counts[c]

    np8 = (np.arange(PADW) < counts[8]).astype(np.float32)
    np9 = (np.arange(PADW) < counts[9]).astype(np.float32)
    npBC = np.concatenate([np8, np9])

    in_maps = []
    for k in range(N_CORES):
        oh3 = np.zeros((NSEG * 128, 3), dtype=np.float32)
        oh3[:counts[k], 0] = 1.0
        oh3[PADW:PADW + 128, 1] = np8[k * 128:(k + 1) * 128]
        oh3[PADW + 128:PADW + 256, 2] = np9[k * 128:(k + 1) * 128]
        rowsBC = np.concatenate([blocks[8, k * 128:(k + 1) * 128],
                                 blocks[9, k * 128:(k + 1) * 128]], axis=0)
        rc3 = np.array([1.0 / counts[k], 1.0 / counts[8], 1.0 / counts[9]],
                       dtype=np.float32)
        in_maps.append({
            "colsA": blocks[k],
            "colsB": blocks[8],
            "colsC": blocks[9],
            "rowsBC": np.ascontiguousarray(rowsBC),
            "oh3": oh3,
            "npBC": npBC,
            "rc3": rc3,
        })
    return in_maps, counts


def finish_loss(cent, spread_sums, local_sums, counts,
                ref_centroid_distances, ref_centroid_angles,
                ref_spreads, ref_local):
    """Host-side final scalar combination (all tiny 10x* tensors), float64."""
    cent = cent.astype(np.float64)
    counts = counts.astype(np.float64)
    spreads = spread_sums.astype(np.float64) / counts
    local = local_sums.astype(np.float64) / counts[:, None]

    # inter-class distances / angles
    sq = np.sum(cent * cent, axis=1)
    d2 = sq[:, None] + sq[None, :] - 2.0 * cent @ cent.T
    d2 = np.maximum(d2, 0.0)
    dists = np.sqrt(d2)
    np.fill_diagonal(dists, 0.0)
    centered = cent - cent.mean(axis=0, keepdims=True)
    nrm = np.linalg.norm(centered, axis=1, keepdims=True)
    normalized = centered / np.maximum(nrm, EPS)
    angles = normalized @ normalized.T

    rcd = np.asarray(ref_centroid_distances, np.float64)
    rca = np.asarray(ref_centroid_angles, np.float64)
    rsp = np.asarray(ref_spreads, np.float64)
    rlo = np.asarray(ref_local, np.float64)

    def mse(a, b):
        return np.mean((a - b) ** 2)

    ref_dist_norm = rcd / (rcd.mean() + EPS)
    curr_dist_norm = dists / (dists.mean() + EPS)
    total = CENTROID_W * (mse(curr_dist_norm, ref_dist_norm) + mse(angles, rca))
    total = total + SPREAD_W * mse(spreads / (spreads.mean() + EPS),
                                   rsp / (rsp.mean() + EPS))
    total = total + LOCAL_W * mse(local, rlo)
    return np.float32(total)


def assemble(results, counts):
    """Gather per-core outputs -> (cent [10,256], spread_sums [10], local_sums [10,10])."""
    cent = np.zeros((C, D), dtype=np.float64)
    spread_sums = np.zeros(C, dtype=np.float64)
    local_sums = np.zeros((C, K), dtype=np.float64)
    for k in range(N_CORES):
        r = results[k]
        cent[k] = r["out_cent"][0]
        spread_sums[k] += r["out_spread"][0, 0]
        local_sums[k] += r["out_local"][0]
        spread_sums[8] += r["out_spread"][1, 0]
        spread_sums[9] += r["out_spread"][2, 0]
        local_sums[8] += r["out_local"][1]
        local_sums[9] += r["out_local"][2]
    cent[8] = results[0]["out_cent"][1]
    cent[9] = results[0]["out_cent"][2]
    return cent, spread_sums, local_sums


def run_on_hw(in_maps, trace=False):
    nc = _get_program()
    res = bass_utils.run_bass_kernel_spmd(
        nc, in_maps, core_ids=list(range(N_CORES)), trace=trace)
    return res


def kernel(embeddings, labels, ref_centroid_distances, ref_centroid_angles,
           ref_spreads, ref_local):
    in_maps, counts = make_core_inputs(embeddings, labels)
    res = run_on_hw(in_maps)
    cent, spread_sums, local_sums = assemble(res.results, counts)
    return finish_loss(cent, spread_sums, local_sums, counts,
                       ref_centroid_distances, ref_centroid_angles,
                       ref_spreads, ref_local)


# revision 13
# speedup vs baseline: 23.6659x; 23.6659x over previous
"""Trainium2 Bass kernel for nn_ClassConditionalCurvature (segment_reduce).

Strategy (8 NeuronCores, SPMD, one data-independent program):
  Host: sort rows by class label, pad every class block to 1024 rows with
  marker embeddings [1e18, 0, ...] (their huge squared norm self-masks them
  out of the top-k, and their one-hot rows are zero so they contribute
  nothing to any segment sum).  Core k owns class k's full 1024-row block
  (k = 0..7) plus chunk k (128 rows) of classes 8 and 9.  Because each core
  holds the *full* column blocks of exactly the classes its rows belong to,
  every per-row quantity (intra-class kNN distances, distance-to-centroid)
  is computed entirely locally -- no collectives.
  Device per 128-row segment: y = -d^2 = 2*(G - sqn/2) - sqm via PE matmuls
  (column norms folded in as a K=1 ones-row matmul), 11 smallest distances
  via max8 -> match_replace -> max8 on the vector engine, sqrt on just the
  10 kept values, then per-class segment sums via small one-hot matmuls.
  Host: assemble the 10x256 centroids / 10 spreads / 10x10 local stats from
  the per-core partials and evaluate the tiny scalar loss terms in numpy.
"""

import numpy as np

import concourse.bacc as bacc
import concourse.bass as bass
import concourse.mybir as mybir
import concourse.tile as tile
from concourse import bass_utils
from concourse.masks import make_identity

F32 = mybir.dt.float32
AF = mybir.ActivationFunctionType
ALU = mybir.AluOpType
AX = mybir.AxisListType

N_CORES = 8
N, D = 8192, 256
C = 10
K = 10  # neighbors
EPS = 1e-8
CENTROID_W, SPREAD_W, LOCAL_W = 1.0, 0.5, 0.5

PADW = 1024          # padded class-block width (rows/cols per class)
NCH = PADW // 128    # chunks per block = 8
NSEG = 10            # row segments per core: 8 (block A) + 1 (B chunk) + 1 (C chunk)
PAD_MARK = 2.0e17    # pad embedding: [PAD_MARK, 0, ..., 0]; pad d^2 ~ 4e34 stays
                     # inside the scalar-engine sqrt domain [0, 2^118]
NEG_BIG = -3.0e38    # match_replace fill


def build_program():
    nc = bacc.Bacc("TRN2", target_bir_lowering=False, debug=False,
                   num_devices=N_CORES)
    # ---- I/O ----
    colsA = nc.dram_tensor("colsA", [PADW, D], F32, kind="ExternalInput")
    colsB = nc.dram_tensor("colsB", [PADW, D], F32, kind="ExternalInput")
    colsC = nc.dram_tensor("colsC", [PADW, D], F32, kind="ExternalInput")
    rowsBC = nc.dram_tensor("rowsBC", [256, D], F32, kind="ExternalInput")
    oh3_d = nc.dram_tensor("oh3", [NSEG * 128, 3], F32, kind="ExternalInput")
    npBC_d = nc.dram_tensor("npBC", [2 * PADW], F32, kind="ExternalInput")
    rc3_d = nc.dram_tensor("rc3", [3], F32, kind="ExternalInput")
    out_cent = nc.dram_tensor("out_cent", [3, D], F32, kind="ExternalOutput")
    out_spread = nc.dram_tensor("out_spread", [3, 1], F32, kind="ExternalOutput")
    out_local = nc.dram_tensor("out_local", [3, K], F32, kind="ExternalOutput")
    # 4-byte passthrough used to chain NEFF executions for timing
    tick = nc.dram_tensor("tick", [1, 1], F32, kind="ExternalInput")
    tock = nc.dram_tensor("tock", [1, 1], F32, kind="ExternalOutput")

    blocks = [colsA, colsB, colsC]

    with tile.TileContext(nc) as tc:
        with (
            tc.tile_pool(name="const", bufs=1) as const,
            tc.tile_pool(name="embt", bufs=1) as embt,
            tc.tile_pool(name="chunks", bufs=4) as chunks,
            tc.tile_pool(name="scratch", bufs=2) as scratch,
            tc.tile_pool(name="ybuf", bufs=2) as ybuf,
            tc.tile_pool(name="small", bufs=3) as small,
            tc.tile_pool(name="tpp", bufs=2, space="PSUM") as tpp,
            tc.tile_pool(name="csump", bufs=1, space="PSUM") as csump,
            tc.tile_pool(name="centtp", bufs=1, space="PSUM") as centtp,
            tc.tile_pool(name="accp", bufs=1, space="PSUM") as accp,
            tc.tile_pool(name="dotp", bufs=1, space="PSUM") as dotp,
            tc.tile_pool(name="gp", bufs=2, space="PSUM") as gp,
        ):
            # ---- constants ----
            ident = const.tile([128, 128], F32)
            make_identity(nc, ident[:])
            ones = const.tile([1, 128], F32)
            nc.vector.memset(ones[:], 1.0)
            eps_t = const.tile([128, 1], F32)
            nc.vector.memset(eps_t[:], EPS)
            oh3 = const.tile([128, NSEG, 3], F32)
            nc.sync.dma_start(oh3[:], oh3_d.ap().rearrange("(c p) f -> p c f", p=128))
            npBC = const.tile([128, 2, NCH], F32)
            nc.sync.dma_start(npBC[:], npBC_d.ap().rearrange("(b c p) -> p b c", b=2, p=128))
            rc3 = const.tile([1, 3], F32)
            nc.sync.dma_start(rc3[:], rc3_d.ap()[None, :])
            tk = const.tile([1, 1], F32)
            nc.sync.dma_start(tk[:], tick.ap())
            nc.sync.dma_start(tock.ap(), tk[:])

            # persistent sbuf tensors
            embT = [embt.tile([128, 2 * PADW], F32, tag=f"embT{x}",
                              name=f"embT{x}") for x in range(3)]
            sqT = [embt.tile([1, PADW], F32, tag=f"sqT{x}", name=f"sqT{x}")
                   for x in range(3)]
            sqB = [embt.tile([128, NCH], F32, tag=f"sqB{x}", name=f"sqB{x}")
                   for x in range(3)]
            sqR = embt.tile([128, 2], F32)          # rowsBC squared norms
            embT_rows = embt.tile([128, 4, 128], F32)  # [p, half*2+j, f]
            negsq = embt.tile([128, NSEG], F32)
            cents = embt.tile([1, 3, D], F32)
            csq3 = embt.tile([1, 3], F32)
            k3 = embt.tile([1, 3], F32)
            centT = embt.tile([128, 6], F32)        # cols 0-2: d<128, 3-5: d>=128

            # PSUM accumulators (one open accumulation group per bank)
            csum_p = csump.tile([1, 256], F32)     # per-block centroid sum (3 sequential groups)
            centT_p = centtp.tile([128, 6], F32)   # 6 single-matmul transposed-centroid cols
            acc_p = accp.tile([3, 1 + K], F32)     # col 0: spread sums, cols 1..10: local sums

            # ---- setup: stream blocks, transpose, norms, centroid sums ----
            for bx in range(3):
                for ci in range(NCH):
                    ch = chunks.tile([128, D], F32)
                    nc.sync.dma_start(ch[:], blocks[bx].ap()[ci * 128:(ci + 1) * 128, :])
                    sc = scratch.tile([128, D], F32)
                    nc.scalar.activation(sc[:], ch[:], AF.Square,
                                         accum_out=sqB[bx][:, ci:ci + 1])
                    tp = tpp.tile([128, 384], F32)
                    nc.tensor.transpose(tp[:, 0:128], ch[:, 0:128], ident[:])
                    nc.tensor.transpose(tp[:, 128:256], ch[:, 128:256], ident[:])
                    nc.tensor.transpose(tp[0:1, 256:384], sqB[bx][:, ci:ci + 1], ident[:])
                    # psum -> embT (lo half to cols [ci*128], hi half to [PADW + ci*128])
                    ev = embT[bx][:].rearrange("p (h w) -> p h w", h=2)
                    nc.vector.tensor_copy(
                        ev[:, :, ci * 128:(ci + 1) * 128],
                        tp[:, 0:256].rearrange("p (h w) -> p h w", h=2))
                    nc.scalar.activation(sqT[bx][0:1, ci * 128:(ci + 1) * 128],
                                         tp[0:1, 256:384], AF.Copy, scale=-0.5)
                    # centroid partial sum over non-pad rows
                    if bx == 0:
                        notpad = oh3[:, ci, 0:1]
                    else:
                        notpad = npBC[:, bx - 1, ci:ci + 1]
                    nc.tensor.matmul(csum_p[:], notpad, ch[:],
                                     start=(ci == 0), stop=(ci == NCH - 1))
                # centroid for this block
                cent_b = cents[:, bx, :]
                nc.scalar.activation(cent_b, csum_p[:], AF.Copy,
                                     scale=rc3[0:1, bx:bx + 1])
                nc.sync.dma_start(out_cent.ap()[bx:bx + 1, :], cent_b)
                sc2 = scratch.tile([1, D], F32, tag="sc2")
                nc.scalar.activation(sc2[:], cent_b, AF.Square,
                                     accum_out=csq3[0:1, bx:bx + 1])
                nc.tensor.matmul(centT_p[:, bx:bx + 1],
                                 cents[0:1, bx, 0:128], ones[0:1, 0:1],
                                 start=True, stop=True)
                nc.tensor.matmul(centT_p[:, 3 + bx:4 + bx],
                                 cents[0:1, bx, 128:256], ones[0:1, 0:1],
                                 start=True, stop=True)

            # rowsBC chunks (rows of classes 8/9 owned by this core)
            for j in range(2):
                ch = chunks.tile([128, D], F32)
                nc.sync.dma_start(ch[:], rowsBC.ap()[j * 128:(j + 1) * 128, :])
                sc = scratch.tile([128, D], F32)
                nc.scalar.activation(sc[:], ch[:], AF.Square,
                                     accum_out=sqR[:, j:j + 1])
                tp = tpp.tile([128, 384], F32)
                nc.tensor.transpose(tp[:, 0:128], ch[:, 0:128], ident[:])
                nc.tensor.transpose(tp[:, 128:256], ch[:, 128:256], ident[:])
                nc.vector.tensor_copy(embT_rows[:, j, :], tp[:, 0:128])
                nc.vector.tensor_copy(embT_rows[:, 2 + j, :], tp[:, 128:256])

            nc.scalar.mul(k3[:], csq3[:], -0.5)
            nc.vector.tensor_copy(centT[:], centT_p[:])
            nc.scalar.mul(negsq[:, 0:NCH], sqB[0][:], -1.0)
            nc.scalar.mul(negsq[:, NCH:NSEG], sqR[:], -1.0)

            # ---- main loop: one 128-row segment at a time ----
            for seg in range(NSEG):
                if seg < NCH:
                    lo = embT[0][:, seg * 128:(seg + 1) * 128]
                    hi = embT[0][:, PADW + seg * 128:PADW + (seg + 1) * 128]
                    cT, sT = embT[0], sqT[0]
                    sqrow = sqB[0][:, seg:seg + 1]
                else:
                    j = seg - NCH
                    lo = embT_rows[:, j, :]
                    hi = embT_rows[:, 2 + j, :]
                    cT, sT = embT[1 + j], sqT[1 + j]
                    sqrow = sqR[:, j:j + 1]
                ohs = oh3[:, seg, :]

                y = ybuf.tile([128, PADW], F32)
                for h in range(2):
                    fs = h * 512
                    g = gp.tile([128, 512], F32)
                    nc.tensor.matmul(g[:], lo, cT[:, fs:fs + 512],
                                     start=True, stop=False)
                    nc.tensor.matmul(g[:], hi, cT[:, PADW + fs:PADW + fs + 512],
                                     start=False, stop=False)
                    nc.tensor.matmul(g[:], ones[:], sT[0:1, fs:fs + 512],
                                     start=False, stop=True)
                    # y = 2*G - sqm   (= -d^2 since G already holds e.e - sqn/2)
                    nc.scalar.activation(y[:, fs:fs + 512], g[:], AF.Identity,
                                         bias=negsq[:, seg:seg + 1], scale=2.0)

                m1 = small.tile([128, 8], F32, tag="m1")
                nc.vector.max(m1[:], y[:])
                nc.vector.match_replace(y[:], m1[:], y[:], NEG_BIG)
                m2 = small.tile([128, 8], F32, tag="m2")
                nc.vector.max(m2[:], y[:])

                knn = small.tile([128, K], F32, tag="knn")
                nc.scalar.activation(knn[:, 0:7], m1[:, 1:8], AF.Sqrt, scale=-1.0)
                nc.scalar.activation(knn[:, 7:10], m2[:, 0:3], AF.Sqrt, scale=-1.0)
                ksum = small.tile([128, 1], F32, tag="ksum")
                nc.vector.reduce_sum(ksum[:], knn[:], axis=AX.X)
                meps = small.tile([128, 1], F32, tag="meps")
                nc.scalar.activation(meps[:], ksum[:], AF.Identity,
                                     bias=eps_t[:], scale=1.0 / K)
                rmean = small.tile([128, 1], F32, tag="rmean")
                nc.vector.reciprocal(rmean[:], meps[:])
                # knd: col 0 = d2c, cols 1..10 = knn_norm -> one segment-sum matmul
                knd = small.tile([128, 1 + K], F32, tag="knd")
                nc.vector.tensor_scalar_mul(knd[:, 1:1 + K], knn[:], rmean[:])

                dt = dotp.tile([128, 3], F32)
                nc.tensor.matmul(dt[:], lo, centT[:, 0:3], start=True, stop=False)
                nc.tensor.matmul(dt[:], hi, centT[:, 3:6], start=False, stop=False)
                nc.tensor.matmul(dt[:], ones[:], k3[:], start=False, stop=True)
                # (tensor_tensor_reduce hard-faults TRN2 here; use mul+reduce)
                scr3 = small.tile([128, 3], F32, tag="scr3")
                sel = small.tile([128, 1], F32, tag="sel")
                nc.vector.tensor_mul(scr3[:], ohs, dt[:])
                nc.vector.reduce_sum(sel[:], scr3[:], axis=AX.X)
                nc.scalar.activation(knd[:, 0:1], sel[:], AF.Sqrt,
                                     bias=sqrow, scale=-2.0)

                nc.tensor.matmul(acc_p[:], ohs, knd[:],
                                 start=(seg == 0), stop=(seg == NSEG - 1))

            # ---- epilogue ----
            acc_sb = small.tile([3, 1 + K], F32, tag="acc_sb")
            nc.vector.tensor_copy(acc_sb[:], acc_p[:])
            nc.sync.dma_start(out_spread.ap(), acc_sb[:, 0:1])
            nc.sync.dma_start(out_local.ap(), acc_sb[:, 1:1 + K])

    nc.compile()
    return nc


_NC_CACHE = None


def _get_program():
    global _NC_CACHE
    if _NC_CACHE is None:
        _NC_CACHE = build_program()
    return _NC_CACHE


def make_core_inputs(embeddings, labels):
    """Host-side sharding: returns (in_maps list per core, counts)."""
    emb = np.ascontiguousarray(np.asarray(embeddings), dtype=np.float32)
    lab = np.asarray(labels).astype(np.int64).ravel()
    n = emb.shape[0]
    counts = np.bincount(lab, minlength=C)
    assert counts.max() <= PADW, f"class too large: {counts.max()} > {PADW}"
    assert counts.min() >= K + 2, f"class too small for kNN: {counts.min()}"

    blocks = np.zeros((C, PADW, D), dtype=np.float32)
    blocks[:, :, 0] = PAD_MARK
    order = np.argsort(lab, kind="stable")
    off = 0
    for c in range(C):
        idx = order[off:off + counts[c]]
        blocks[c, :counts[c]] = emb[idx]
        off += counts[c]

    np8 = (np.arange(PADW) < counts[8]).astype(np.float32)
    np9 = (np.arange(PADW) < counts[9]).astype(np.float32)
    npBC = np.concatenate([np8, np9])

    in_maps = []
    for k in range(N_CORES):
        oh3 = np.zeros((NSEG * 128, 3), dtype=np.float32)
        oh3[:counts[k], 0] = 1.0
        oh3[PADW:PADW + 128, 1] = np8[k * 128:(k + 1) * 128]
        oh3[PADW + 128:PADW + 256, 2] = np9[k * 128:(k + 1) * 128]
        rowsBC = np.concatenate([blocks[8, k * 128:(k + 1) * 128],
                                 blocks[9, k * 128:(k + 1) * 128]], axis=0)
        rc3 = np.array([1.0 / counts[k], 1.0 / counts[8], 1.0 / counts[9]],
                       dtype=np.float32)
        in_maps.append({
            "colsA": blocks[k],
            "colsB": blocks[8],
            "colsC": blocks[9],
            "rowsBC": np.ascontiguousarray(rowsBC),
            "oh3": oh3,
            "npBC": npBC,
            "rc3": rc3,
            "tick": np.zeros((1, 1), dtype=np.float32),
        })
    return in_maps, counts


def finish_loss(cent, spread_sums, local_sums, counts,
                ref_centroid_distances, ref_centroid_angles,
                ref_spreads, ref_local):
    """Host-side final scalar combination (all tiny 10x* tensors), float64."""
    cent = cent.astype(np.float64)
    counts = counts.astype(np.float64)
    spreads = spread_sums.astype(np.float64) / counts
    local = local_sums.astype(np.float64) / counts[:, None]

    # inter-class distances / angles
    sq = np.sum(cent * cent, axis=1)
    d2 = sq[:, None] + sq[None, :] - 2.0 * cent @ cent.T
    d2 = np.maximum(d2, 0.0)
    dists = np.sqrt(d2)
    np.fill_diagonal(dists, 0.0)
    centered = cent - cent.mean(axis=0, keepdims=True)
    nrm = np.linalg.norm(centered, axis=1, keepdims=True)
    normalized = centered / np.maximum(nrm, EPS)
    angles = normalized @ normalized.T

    rcd = np.asarray(ref_centroid_distances, np.float64)
    rca = np.asarray(ref_centroid_angles, np.float64)
    rsp = np.asarray(ref_spreads, np.float64)
    rlo = np.asarray(ref_local, np.float64)

    def mse(a, b):
        return np.mean((a - b) ** 2)

    ref_dist_norm = rcd / (rcd.mean() + EPS)
    curr_dist_norm = dists / (dists.mean() + EPS)
    total = CENTROID_W * (mse(curr_dist_norm, ref_dist_norm) + mse(angles, rca))
    total = total + SPREAD_W * mse(spreads / (spreads.mean() + EPS),
                                   rsp / (rsp.mean() + EPS))
    total = total + LOCAL_W * mse(local, rlo)
    return np.float32(total)


def assemble(results, counts):
    """Gather per-core outputs -> (cent [10,256], spread_sums [10], local_sums [10,10])."""
    cent = np.zeros((C, D), dtype=np.float64)
    spread_sums = np.zeros(C, dtype=np.float64)
    local_sums = np.zeros((C, K), dtype=np.float64)
    for k in range(N_CORES):
        r = results[k]
        cent[k] = r["out_cent"][0]
        spread_sums[k] += r["out_spread"][0, 0]
        local_sums[k] += r["out_local"][0]
        spread_sums[8] += r["out_spread"][1, 0]
        spread_sums[9] += r["out_spread"][2, 0]
        local_sums[8] += r["out_local"][1]
        local_sums[9] += r["out_local"][2]
    cent[8] = results[0]["out_cent"][1]
    cent[9] = results[0]["out_cent"][2]
    return cent, spread_sums, local_sums


def run_on_hw(in_maps, trace=False):
    nc = _get_program()
    res = bass_utils.run_bass_kernel_spmd(
        nc, in_maps, core_ids=list(range(N_CORES)), trace=trace)
    return res


def kernel(embeddings, labels, ref_centroid_distances, ref_centroid_angles,
           ref_spreads, ref_local):
    in_maps, counts = make_core_inputs(embeddings, labels)
    res = run_on_hw(in_maps)
    cent, spread_sums, local_sums = assemble(res.results, counts)
    return finish_loss(cent, spread_sums, local_sums, counts,
                       ref_centroid_distances, ref_centroid_angles,
                       ref_spreads, ref_local)
